# revision 10
# baseline (speedup 1.0000x reference)
"""Squared Euclidean distance matrix kernel for Trainium2 (8 NeuronCores).

out[i, j] = ||mat_1[i] - mat_2[j]||^2 = sq1[i] + sq2[j] - 2 * mat_1[i].mat_2[j]

Sharding: rows of mat_1 (= rows of the output) split across 8 cores;
mat_2 replicated. Each core computes a [1024, 8192] tile of the output.

Per-core dataflow (cross GEMM in bf16; output written fp16, upcast on host —
fp16 quantization adds ~2e-3 vs the 2e-2 gate; fp8 was tried and measured:
DoubleRow gives NO stream speedup at K=128, the PE stream is column-rate
limited):
  - Host pre-transposes inputs so the contraction dim (d=128) lands on SBUF
    partitions and folds the -2 scale into m1ts; also ships m1 in natural
    chunk layout [128, 8, 128] for the per-partition sq1 column.
  - sq1 row + sq2 row: squares on DVE, colsums via ones-matmuls on PE
    (shifted one-hot stationary Woh lands each chunk in its own psum
    partition so a batch drains with ONE [8,512] copy). sq1 column
    [128, n_mb] f32 via DVE mul + axis-X tensor_reduce on the natural
    layout. SQ2B = sq2 broadcast to all 128 partitions via a log2 DMA
    doubling ladder (DMA is the only partition-crossing mover).
  - Main loop per (g, mi) unit: 4 psum tiles [128, 1024] (2 banks each):
      all:   psum = m1ts.T @ m2t  (K=128 bf16, -2*cross, 2 matmuls/tile)
      T0,T2: psum += [ones; sq1].T @ [sq2; ones] (K=2 fp16 matmul)
             then plain ScalarE copy -> fp16 staging
      T1,T3: VectorE scalar_tensor_tensor: (psum + sq1_col) + SQ2B -> fp16
             (no K=2 matmul -- the rank-1 terms ride the psum drain)
    This halves the PE's rank-2 matmul work; PE ~12 streams/unit ~= the
    16.8 MB fp16 output's DMA floor (~42us at ~400 GB/s/core).
  - Column-half g runs OUTER so g=0 needs only the first sq2 batch; the
    second input half streams in under the main loop. DMA issue order is
    arranged so early-needed chunks get full bandwidth (later chunk loads
    queue behind a data-dependent DMA, which stalls that queue until the
    sq1 row is ready).
  - Dummy Woh matmuls bridge the PE-idle window while sq2 squares wait on
    input DMA: the HAM clock gate re-throttles the PE to 1.2 GHz after
    idle, and warm (2.4 GHz) vs cold is 2x on every matmul.
"""

import sys

import numpy as np

if "/opt/trn_rl_repo" not in sys.path:
    sys.path.insert(0, "/opt/trn_rl_repo")

import concourse.bass as bass
import concourse.mybir as mybir
import concourse.tile as tile
from concourse.bass_utils import run_bass_kernel_spmd

N1, N2, D = 8192, 8192, 128
NCORES = 8
MS = N1 // NCORES  # 1024 output rows per core

F32 = mybir.dt.float32
BF16 = mybir.dt.bfloat16
F16 = mybir.dt.float16
ALU = mybir.AluOpType
AXIS = mybir.AxisListType


def legalize_waits(nc):
    """Split multi-wait instructions into single-wait NoOps.

    The TPB ISA encodes exactly one sync-wait per instruction and this
    walrus build refuses instructions carrying more. Semantics are preserved
    by having the same engine execute one NoOp per extra wait immediately
    before the instruction.
    """
    n = 0
    for fn in nc.m.functions:
        for blk in fn.blocks:
            new_list = []
            changed = False
            for inst in blk.instructions:
                si = inst.sync_info
                waits = list(si.on_wait) if si and si.on_wait else []
                if len(waits) > 1:
                    changed = True
                    for w in waits[:-1]:
                        nop = mybir.InstNoOp(name=f"I-wsplit-{n}", ins=[], outs=[])
                        n += 1
                        nop.engine = inst.engine
                        nop.sync_info = mybir.SyncInfo(on_wait=[w], on_update=[])
                        new_list.append(nop)
                    si.on_wait = [waits[-1]]
                    inst.sync_info = si
                new_list.append(inst)
            if changed:
                blk.instructions = new_list
    return nc


def build_nc(ms=MS, n2=N2, d=D, legalize=True):
    """Build the per-core Bass module. All cores run the same program (SPMD);
    the mat_1 shard differs per core via in_maps."""
    assert ms % 512 == 0 and n2 % 4096 == 0 and d == 128
    n_mb = ms // 128    # M blocks of 128 rows (8)
    n_nb = n2 // 512    # N blocks of 512 cols (16)
    n_g = n_nb // 8     # column-half units per mi (2)

    nc = bass.Bass()
    m1ts = nc.declare_dram_parameter("m1ts", [d, ms], BF16, isOutput=False)
    m1nat = nc.declare_dram_parameter("m1nat", [128, n_mb, d], BF16, isOutput=False)
    m2t = nc.declare_dram_parameter("m2t", [d, n2], BF16, isOutput=False)
    out = nc.declare_dram_parameter("out", [ms, n2], F16, isOutput=True)

    with tile.TileContext(nc) as tc:
        with (
            tc.tile_pool(name="big", bufs=1) as big,
            tc.tile_pool(name="scratch", bufs=2) as scr,
            tc.tile_pool(name="sqst", bufs=3) as sqstp,
            tc.tile_pool(name="stage", bufs=3) as stagep,
            tc.tile_pool(name="psum", bufs=4, space="PSUM") as psump,
        ):
            # ---- early input loads (full-bandwidth set: everything the
            #      preamble needs; c2/c3 are issued later, queued behind a
            #      data-dependent DMA so they don't steal bandwidth now) ----
            M1TS = big.tile([d, ms], BF16, tag="m1ts")
            M1NAT = big.tile([128, n_mb, d], BF16, tag="m1nat")
            M2T = big.tile([d, n2], BF16, tag="m2t")
            nc.sync.dma_start(out=M1TS[:], in_=m1ts[:])
            nc.sync.dma_start(out=M1NAT[:], in_=m1nat[:])
            nc.sync.dma_start(out=M2T[:, 0:2048], in_=m2t[:, 0:2048])
            nc.scalar.dma_start(out=M2T[:, 2048:4096], in_=m2t[:, 2048:4096])

            # ---- constants ----
            onesA = big.tile([128, 64], F16, tag="onesA")
            nc.vector.memset(onesA[:], 1.0)
            # Shifted one-hot stationary: Woh[:, 8] = 1, rest 0. sq-matmul c
            # uses lhsT = Woh[:, 8-c : 16-c] so its colsum lands in partition c.
            Woh = big.tile([128, 17], F16, tag="woh")
            nc.vector.memset(Woh[:], 0.0)
            nc.vector.memset(Woh[:, 8:9], 1.0)

            LHS2 = big.tile([2, ms], F16, tag="lhs2")   # [ones; sq1] rows
            nc.sync.dma_start(out=LHS2[0:1, :], in_=onesA[:, 0 : ms // 128])
            RHS2 = big.tile([2, n2], F16, tag="rhs2")   # [sq2; ones] rows
            nc.sync.dma_start(out=RHS2[1:2, :], in_=onesA[:, 0 : n2 // 128])
            SQ1C = big.tile([128, n_mb], F32, tag="sq1c")  # sq1 column layout
            SQ2B = big.tile([128, n2], F16, tag="sq2b")    # sq2 bcast to 128p

            # ---- sq1 (row via ones-matmul; column via axis-X reduce) ----
            n_c1 = ms // 512
            sq1_scr = scr.tile([d, ms], F16, tag="sq1_scr")
            nc.vector.tensor_mul(sq1_scr[:], M1TS[:], M1TS[:])
            ps_sq1 = psump.tile([8, 512], F32, tag="ps")
            for c in range(n_c1):
                nc.tensor.matmul(
                    ps_sq1[:],
                    Woh[:, 8 - c : 16 - c],
                    sq1_scr[:, c * 512 : (c + 1) * 512],
                    start=(c == 0),
                    stop=(c == n_c1 - 1),
                )
            sq1_st = sqstp.tile([8, 512], F16, tag="sq1_st")
            nc.scalar.mul(sq1_st[:n_c1, :], ps_sq1[:n_c1, :], 0.25)
            nc.sync.dma_start(out=LHS2[1:2, :], in_=sq1_st[:n_c1, :])
            # column: sq1c[p, mi] = sum_d m1nat[p, mi, d]^2  (natural layout)
            nat_sq = scr.tile([128, n_mb, d], F16, tag="nat_sq")
            nc.vector.tensor_mul(nat_sq[:], M1NAT[:], M1NAT[:])
            nc.vector.tensor_reduce(SQ1C[:], nat_sq[:], axis=AXIS.X, op=ALU.add)

            # ---- dummy warm-up matmuls: keep the PE busy (HAM warm) while
            #      the sq2 squares wait on the m2t chunk DMAs ----
            ps_dummy = psump.tile([8, 512], F32, tag="ps")
            for _w in range(10):
                nc.tensor.matmul(
                    ps_dummy[:],
                    Woh[:, 8:16],
                    sq1_scr[:, 0:512],
                    start=True,
                    stop=True,
                    skip_group_check=True,
                )

            def sq2_batch(b, copy_engine):
                """sq2 for columns [b*4096, (b+1)*4096) -> RHS2 row 0 staging."""
                sq_scr = scr.tile([d, 4096], F16, tag="sq2_scr")
                for k in range(2):
                    c0 = b * 4096 + k * 2048
                    nc.vector.tensor_mul(
                        sq_scr[:, k * 2048 : (k + 1) * 2048],
                        M2T[:, c0 : c0 + 2048],
                        M2T[:, c0 : c0 + 2048],
                    )
                ps_b = psump.tile([8, 512], F32, tag="ps")
                for c in range(8):
                    nc.tensor.matmul(
                        ps_b[:],
                        Woh[:, 8 - c : 16 - c],
                        sq_scr[:, c * 512 : (c + 1) * 512],
                        start=(c == 0),
                        stop=(c == 7),
                    )
                st_b = sqstp.tile([8, 512], F16, tag="sq2_st")
                copy_engine(st_b[:], ps_b[:])
                return st_b

            def sq2_row_and_bcast(b, st_b, between=None):
                """st_b -> RHS2 row segment, then ladder-broadcast into SQ2B.
                `between` emits extra Sync-queue DMAs after the (stalling)
                RHS2 row write and before the ladder."""
                c0 = b * 4096
                nc.sync.dma_start(out=RHS2[0:1, c0 : c0 + 4096], in_=st_b[:])
                if between is not None:
                    between()
                nc.sync.dma_start(out=SQ2B[0:1, c0 : c0 + 4096], in_=st_b[:])
                p = 1
                while p < 128:
                    nc.sync.dma_start(
                        out=SQ2B[p : 2 * p, c0 : c0 + 4096],
                        in_=SQ2B[0:p, c0 : c0 + 4096],
                    )
                    p *= 2

            def load_second_half():
                nc.sync.dma_start(out=M2T[:, 4096:6144], in_=m2t[:, 4096:6144])
                nc.sync.dma_start(out=M2T[:, 6144:8192], in_=m2t[:, 6144:8192])

            # sq2 batch 0 (cols 0:4096). Batch 1 runs under the main loop.
            # The second-half input loads queue right behind batch 0's
            # (data-stalled) RHS2 row DMA: they get no bandwidth until the
            # preamble's critical transfers are done, then run at full rate.
            st_b0 = sq2_batch(0, nc.vector.tensor_copy)
            sq2_row_and_bcast(0, st_b0, between=load_second_half)

            def unit(g, mi, act_only=False, split_out=False):
                """One (g, mi) unit: 8 nj columns, 4 psum tiles [128,1024]."""
                r0 = mi * 128
                acts = tuple(range(4)) if act_only else (0, 2)
                pss = []
                for t in range(4):
                    ps = psump.tile([128, 1024], F32, tag="ps")
                    for h in range(2):
                        c0 = (g * 8 + t * 2 + h) * 512
                        nc.tensor.matmul(
                            ps[:, h * 512 : (h + 1) * 512],
                            M1TS[:, r0 : r0 + 128],
                            M2T[:, c0 : c0 + 512],
                            start=True,
                            # STT tiles get no K=2 matmul; their single-matmul
                            # accumulation group must close here.
                            stop=(t not in acts),
                        )
                    pss.append(ps)
                for t in acts:
                    for h in range(2):
                        c0 = (g * 8 + t * 2 + h) * 512
                        nc.tensor.matmul(
                            pss[t][:, h * 512 : (h + 1) * 512],
                            LHS2[:, r0 : r0 + 128],
                            RHS2[:, c0 : c0 + 512],
                            start=False,
                            stop=True,
                        )
                stage = stagep.tile([128, 4096], F16, tag="stage")
                for t in range(4):
                    dst = stage[:, t * 1024 : (t + 1) * 1024]
                    if t in acts:
                        nc.scalar.copy(dst, pss[t][:])
                    else:
                        c0 = (g * 8 + t * 2) * 512
                        nc.vector.scalar_tensor_tensor(
                            dst,
                            pss[t][:],
                            SQ1C[:, mi : mi + 1],
                            SQ2B[:, c0 : c0 + 1024],
                            op0=ALU.add,
                            op1=ALU.add,
                        )
                o0 = g * 4096
                if split_out:  # smaller final transfers shrink the drain tail
                    nc.sync.dma_start(
                        out=out[r0 : r0 + 128, o0 : o0 + 2048], in_=stage[:, 0:2048]
                    )
                    nc.sync.dma_start(
                        out=out[r0 : r0 + 128, o0 + 2048 : o0 + 4096],
                        in_=stage[:, 2048:4096],
                    )
                else:
                    nc.sync.dma_start(
                        out=out[r0 : r0 + 128, o0 : o0 + 4096], in_=stage[:]
                    )

            # mi0 runs ACT-only (all four tiles via the K=2 matmul) so the
            # SQ2B broadcast ladder has time to land before the first
            # scalar_tensor_tensor consumer.
            unit(0, 0, act_only=True)

            # sq2 batch 1; its RHS2/SQ2B DMAs are emitted after mi2's output
            # DMA so they don't stall the Sync queue while waiting on data.
            st_b1 = sq2_batch(1, nc.scalar.copy)
            unit(0, 1)
            unit(0, 2)
            sq2_row_and_bcast(1, st_b1)
            for mi in range(3, n_mb):
                unit(0, mi)
            for g in range(1, n_g):
                for mi in range(n_mb):
                    unit(g, mi, split_out=(g == n_g - 1 and mi == n_mb - 1))
    return legalize_waits(nc) if legalize else nc


_NC_CACHE = {}


def _get_nc(ms=MS, n2=N2, d=D):
    key = (ms, n2, d)
    if key not in _NC_CACHE:
        _NC_CACHE[key] = build_nc(ms, n2, d)
    return _NC_CACHE[key]


def _prep_inputs(m1, m2, ms):
    """Host-side layout/precision prep (transpose + dtype casts only)."""
    bf16 = mybir.dt.np(BF16)
    m1ts = np.ascontiguousarray(-2.0 * m1.T).astype(bf16)  # [128, n1]
    m2t = np.ascontiguousarray(m2.T).astype(bf16)          # [128, n2]
    ncores = m1ts.shape[1] // ms
    n_mb = ms // 128
    maps = []
    for c in range(ncores):
        m1c = m1[c * ms : (c + 1) * ms].astype(bf16)       # [ms, 128]
        m1nat = np.ascontiguousarray(
            m1c.reshape(n_mb, 128, 128).transpose(1, 0, 2)  # [128, n_mb, 128]
        )
        maps.append(
            {
                "m1ts": np.ascontiguousarray(m1ts[:, c * ms : (c + 1) * ms]),
                "m1nat": m1nat,
                "m2t": m2t,
            }
        )
    return maps


def kernel(mat_1, mat_2, _trace=False):
    m1 = np.ascontiguousarray(np.asarray(mat_1, dtype=np.float32))
    m2 = np.ascontiguousarray(np.asarray(mat_2, dtype=np.float32))
    assert m1.shape == (N1, D) and m2.shape == (N2, D)

    in_maps = _prep_inputs(m1, m2, MS)
    nc = _get_nc()
    r = run_bass_kernel_spmd(nc, in_maps, list(range(NCORES)), trace=_trace)
    out = np.concatenate(
        [r.results[c]["out"].astype(np.float32) for c in range(NCORES)], axis=0
    )
    if _trace:
        return out, r.exec_time_ns
    return out
